# revision 57
# baseline (speedup 1.0000x reference)
"""Trainium2 Bass kernel for the ColBERT MaxSim retrieval problem.

Computes, for fixed shapes Q=64,Sq=32,C=256,Sc=256,H=768,D=128:
    q_pooled = l2norm(q_hidden[:,0] * q_mask[:,0])            [Q,H]
    c_pooled = l2norm(c_hidden[:,0] * c_mask[:,0])            [C,H]
    q_col    = l2norm((q_hidden*q_mask)[:,1:] @ W + b)        [Q,31,D]
    c_col    = l2norm((c_hidden*c_mask)[:,1:] @ W + b)        [C,255,D]
    sim[q,c] = sum_i max_j (q_col[q,i] . c_col[c,j]) / sum(q_mask[q,1:])

Sharding: candidates C are split across 8 NeuronCores (32 docs each);
q tensors / W / b are replicated.  Device-side layout is "colT":
projected token vectors live as [D=128 partitions, tokens] so the
similarity matmul contracts over D on the partition axis.

Exact mask-aware packing (host-side marshalling, device does all FLOPs):
a masked token's projected vector is always bhat = l2norm(b) (the
hidden state is zeroed before the linear layer, bias added after).  So
 - c side: per doc keep only unmasked token columns, pad each doc to a
   fixed width W with a masked-token column (which projects to bhat) —
   exactly the candidates the reference max_j sees.  Docs with no
   masked tokens are padded with a repeated real token (max-idempotent).
 - q side: drop masked query-token rows; add ONE shared all-zero row
   (projects to bhat) whose selector weight for query q is
   (31 - k_q), the number of masked tokens, since each of them would
   have contributed max_j(bhat . c_j).
The final segmented sum over query tokens is a small selector matmul.
"""

import os
import sys
import types

import numpy as np

for _p in ("/opt/trn_rl_repo", os.path.expanduser("~/.axon_site/_ro/trn_rl_repo")):
    if os.path.isdir(_p) and _p not in sys.path:
        sys.path.insert(0, _p)

# ---- problem constants (hardcoded per harness contract) ----
Q, SQ, C, SC, H, D = 64, 32, 256, 256, 768, 128
NCORES = 8
CPC = C // NCORES            # 32 docs per core
QTOK = Q * (SQ - 1)          # 1984 query tokens (unpacked)
HC = H // 128                # 6 contraction chunks
PBLK = 512                   # projection block (col count per psum tile)

USE_F32R = True              # fast fp32 matmul mode (1 cycle/row at N>=256)
SIM_BF16 = True              # bf16 operands for the similarity/final matmuls
PROJ_BF16 = True             # bf16 hidden-states/W for the projection matmuls

_PROGS = {}


def _install_ntff_hook():
    try:
        from antenv import axon_hooks  # noqa: F401
        return
    except ImportError:
        pass
    try:
        import antenv
        from trn_agent_boot.trn_boot import _ntff_profile_via_ctypes
    except ImportError:
        return
    mod = types.ModuleType("antenv.axon_hooks")
    holder = [None]
    mod.set_axon_ntff_profile_hook = lambda h: holder.__setitem__(0, h)
    mod.get_axon_ntff_profile_hook = lambda: holder[0]
    sys.modules["antenv.axon_hooks"] = mod
    antenv.axon_hooks = mod
    so = "/opt/axon/libaxon_pjrt.so"
    if os.path.exists(so):
        mod.set_axon_ntff_profile_hook(_ntff_profile_via_ctypes(so))


def _make_plan(ns_sorted, nqtile):
    """Build the shared per-core reduce-group plan from the globally
    DESC-sorted unmasked-token counts (len C).  Core s owns sorted docs
    [s::NCORES]; group widths are set by the group's first (largest)
    doc, which is the global max across cores for that local position.

    Returns params dict with plan = list of groups
    (local_doc0, dpm, nmm, W, ccol_off, slot_off)."""
    plan = []
    i = 0          # core-local doc index (sorted order)
    ccoff = 0
    while i < CPC:
        Wg = int(ns_sorted[i * NCORES]) + 1
        Wg += Wg % 2
        Wg = min(SC - 1, Wg)
        dpm = min(4, 512 // Wg)
        nd = min(2 * dpm, CPC - i)
        nmm = -(-nd // dpm)
        if nmm * dpm != nd:        # keep groups rectangular
            nd = (nd // dpm) * dpm
            nmm = nd // dpm
            if nd == 0:
                dpm = CPC - i
                nd = dpm
                nmm = 1
        plan.append((i, dpm, nmm, Wg, ccoff, i))
        ccoff += nd * Wg
        i += nd
    ctp = ccoff
    ctp_pad = -(-ctp // PBLK) * PBLK
    qtp = nqtile * 128
    key = (nqtile,) + tuple((dpm, nmm, W) for _, dpm, nmm, W, _, _ in plan)
    return dict(plan=plan, ctp=ctp, ctp_pad=ctp_pad, nqt=nqtile, qtp=qtp,
                key=key)


def _build_program(p):
    import concourse.tile as tile
    from concourse import bacc, mybir

    f32 = mybir.dt.float32
    f32m = mybir.dt.float32r if USE_F32R else f32
    simdt = mybir.dt.bfloat16 if SIM_BF16 else f32m
    hdt = mybir.dt.bfloat16 if PROJ_BF16 else f32m
    AF = mybir.ActivationFunctionType
    AX = mybir.AxisListType

    plan, ctp_pad, nqt, qtp = p["plan"], p["ctp_pad"], p["nqt"], p["qtp"]

    nc = bacc.Bacc(
        "TRN2",
        target_bir_lowering=False,
        debug=False,
        enable_asserts=False,
        num_devices=NCORES,
    )

    qT = nc.dram_tensor("qT", [HC, 128, qtp], hdt, kind="ExternalInput").ap()
    cT = nc.dram_tensor("cT", [HC, 128, ctp_pad], hdt, kind="ExternalInput").ap()
    qcls = nc.dram_tensor("qcls", [Q, H], f32, kind="ExternalInput").ap()
    ccls = nc.dram_tensor("ccls", [CPC, H], f32, kind="ExternalInput").ap()
    Wb = nc.dram_tensor("Wb", [HC, 128, D], hdt, kind="ExternalInput").ap()
    bb = nc.dram_tensor("bb", [D, 1], f32, kind="ExternalInput").ap()
    Ssel = nc.dram_tensor("Ssel", [nqt, 128, Q], simdt, kind="ExternalInput").ap()
    invd = nc.dram_tensor("invd", [Q, 1], f32, kind="ExternalInput").ap()
    onesc = nc.dram_tensor("onesc", [128, 1], simdt, kind="ExternalInput").ap()
    onesr = nc.dram_tensor("onesr", [1, 128], f32m, kind="ExternalInput").ap()

    osim = nc.dram_tensor("osim", [Q, CPC], f32, kind="ExternalOutput").ap()
    oqp = nc.dram_tensor("oqp", [Q, H], f32, kind="ExternalOutput").ap()
    ocp = nc.dram_tensor("ocp", [CPC, H], f32, kind="ExternalOutput").ap()

    with tile.TileContext(nc) as tc:
        with tc.tile_pool(name="const", bufs=1) as const:
            w_sb = const.tile([128, HC * D], hdt)
            nc.sync.dma_start(w_sb[:].rearrange("p (k d) -> p k d", k=HC),
                              Wb.rearrange("k p d -> p k d"))
            b_sb = const.tile([D, 1], f32)
            nc.scalar.dma_start(b_sb[:], bb)
            ones_col = const.tile([128, 1], simdt)
            nc.scalar.dma_start(ones_col[:], onesc)
            ones_row = const.tile([1, 128], f32m)
            nc.scalar.dma_start(ones_row[:], onesr)
            s_sb = const.tile([128, nqt * Q], simdt)
            nc.sync.dma_start(s_sb[:].rearrange("p (t q) -> p t q", t=nqt),
                              Ssel.rearrange("t p q -> p t q"))
            invd_sb = const.tile([Q, 1], f32)
            nc.scalar.dma_start(invd_sb[:], invd)

            qcol = const.tile([128, qtp], simdt)
            ccol = const.tile([128, ctp_pad], simdt)
            md = const.tile([128, nqt * CPC], simdt)

            # Flat scope; PSUM budget 8 banks: y 2 + ss 1 + r/fin 1 + sim 4.
            with tc.tile_pool(name="hT", bufs=6) as hpool, \
                 tc.tile_pool(name="zp", bufs=4) as zpool, \
                 tc.tile_pool(name="rows", bufs=4) as rowpool, \
                 tc.tile_pool(name="ypsum", bufs=2, space="PSUM") as ypsum, \
                 tc.tile_pool(name="spsum", bufs=1, space="PSUM") as spsum, \
                 tc.tile_pool(name="rpsum", bufs=1, space="PSUM") as rpsum, \
                 tc.tile_pool(name="simpsum", bufs=2, space="PSUM") as simpsum:

                # PE warm-up burst: dense matmuls during the DMA ramp so the
                # HAM clock-gate releases (K=8/8) before projection work.
                warm = spsum.tile([128, 512], f32, tag="ss")
                for _ in range(16):
                    nc.tensor.matmul(warm[:], w_sb[:, :128], w_sb[:, :512],
                                     start=True, stop=True)

                def project(srcT, ntok, dst):
                    nblk = -(-ntok // PBLK)
                    for i in range(nblk):
                        o = i * PBLK
                        blk = min(PBLK, ntok - o)
                        h_sb = hpool.tile([128, HC, PBLK], hdt, tag="hT")
                        dma_eng = nc.sync if i % 2 == 0 else nc.gpsimd
                        dma_eng.dma_start(
                            h_sb[:, :, :blk],
                            srcT.rearrange("k p t -> p k t")[:, :, o:o + blk])
                        y_ps = ypsum.tile([128, 512], f32, tag="y")
                        for k in range(HC):
                            nc.tensor.matmul(
                                y_ps[:, :blk],
                                w_sb[:, k * D:(k + 1) * D],
                                h_sb[:, k, :blk],
                                start=(k == 0),
                                stop=(k == HC - 1),
                            )
                        # z = y + b  (bias per-partition), PSUM -> SBUF
                        z_sb = zpool.tile([128, PBLK], f32, tag="z")
                        nc.scalar.activation(z_sb[:, :blk], y_ps[:, :blk],
                                             AF.Identity, bias=b_sb[:], scale=1.0)
                        zsq = zpool.tile([128, PBLK], simdt, tag="zsq")
                        nc.vector.tensor_mul(zsq[:, :blk], z_sb[:, :blk],
                                             z_sb[:, :blk])
                        # sumsq over D via ones-matmul -> [1, blk]
                        ss_ps = spsum.tile([1, 512], f32, tag="ss")
                        nc.tensor.matmul(ss_ps[:, :blk], ones_col[:],
                                         zsq[:, :blk], start=True, stop=True)
                        # norm row [1,blk] (single ACT op: PSUM->SBUF + sqrt)
                        nrm = rowpool.tile([1, PBLK], f32m, tag="nrm")
                        nc.scalar.activation(nrm[:, :blk], ss_ps[:, :blk], AF.Sqrt)
                        # broadcast norm across partitions via K=1 matmul, then
                        # full-width reciprocal + multiply (128 lanes, not 1)
                        r_ps = rpsum.tile([128, 512], f32, tag="r")
                        nc.tensor.matmul(r_ps[:, :blk], ones_row[:],
                                         nrm[:, :blk], start=True, stop=True)
                        rcp = zpool.tile([128, PBLK], f32, tag="rcp")
                        nc.vector.reciprocal_approx_fast(rcp[:, :blk], r_ps[:, :blk])
                        nc.gpsimd.tensor_mul(dst[:, o:o + blk], z_sb[:, :blk],
                                              rcp[:, :blk])

                project(qT, qtp, qcol)

                # ---- pooled CLS outputs (natural layout, row-wise l2norm) ----
                def pooled(cls_ap, n, out_ap):
                    tcl = zpool.tile([n, H], f32, tag="cls%d" % n)
                    nc.scalar.dma_start(tcl[:], cls_ap)
                    sq = zpool.tile([n, H], f32, tag="clsq%d" % n)
                    nc.scalar.activation(sq[:], tcl[:], AF.Square)
                    s0 = rowpool.tile([n, 1], f32, tag="clss%d" % n)
                    nc.vector.reduce_sum(s0[:], sq[:], axis=AX.X)
                    s1 = rowpool.tile([n, 1], f32, tag="clss1%d" % n)
                    nc.scalar.activation(s1[:], s0[:], AF.Sqrt)
                    s2 = rowpool.tile([n, 1], f32, tag="clss2%d" % n)
                    nc.vector.tensor_scalar_max(s2[:], s1[:], 1e-12)
                    rr = rowpool.tile([n, 1], f32, tag="clsr%d" % n)
                    nc.vector.reciprocal(rr[:], s2[:])
                    oo = zpool.tile([n, H], f32, tag="clso%d" % n)
                    nc.vector.tensor_scalar_mul(oo[:], tcl[:], rr[:])
                    nc.sync.dma_start(out_ap, oo[:])

                pooled(qcls, Q, oqp)
                pooled(ccls, CPC, ocp)

                project(cT, ctp_pad, ccol)

                # ---- phase B: similarity matmuls + per-doc max ----
                for t in range(nqt):
                    qw = qcol[:, t * 128:(t + 1) * 128]
                    for gi, (i0, dpm, nmm, Wg, ccoff, sloff) in enumerate(plan):
                        if t >= 2 and gi % 2 == 0:
                            # HAM keep-alive: extra array activity so the
                            # clock-gate stays at K=8/8 through the
                            # reduce-bound tail (PE duty alone is too low).
                            ka = ypsum.tile([128, 512], f32, tag="y")
                            nc.tensor.matmul(ka[:, :512], qw,
                                             ccol[:, ccoff:ccoff + 512],
                                             start=True, stop=True)
                        ps = simpsum.tile([128, 1024], f32, tag="sim")
                        mw = dpm * Wg
                        for m in range(nmm):
                            nc.tensor.matmul(
                                ps[:, m * 512:m * 512 + mw], qw,
                                ccol[:, ccoff + m * mw: ccoff + (m + 1) * mw],
                                start=True, stop=True)
                        red_in = (
                            ps[:, :]
                            .rearrange("p (k x) -> p k x", k=2)[:, :nmm, :mw]
                            .rearrange("p k (d j) -> p k d j", j=Wg)
                        )
                        nd = dpm * nmm
                        nc.vector.reduce_max(
                            md[:, t * CPC + sloff: t * CPC + sloff + nd],
                            red_in, axis=AX.X,
                        )

                # ---- phase C: weighted sum over query tokens + scale ----
                fp = spsum.tile([Q, CPC], f32, tag="ss")
                for t in range(nqt):
                    nc.tensor.matmul(
                        fp[:],
                        s_sb[:, t * Q:(t + 1) * Q],
                        md[:, t * CPC: (t + 1) * CPC],
                        start=(t == 0), stop=(t == nqt - 1),
                    )
                so = zpool.tile([Q, CPC], f32, tag="so")
                nc.vector.tensor_scalar_mul(so[:], fp[:], invd_sb[:])
                nc.sync.dma_start(osim, so[:])

    nc.compile()
    return nc


def _get_program(p):
    key = p["key"]
    if key not in _PROGS:
        _PROGS[key] = _build_program(p)
    return _PROGS[key]


def _prepare(q_hidden, c_hidden, W, b, q_mask, c_mask):
    f32 = np.float32
    qm = (q_hidden.astype(f32) * q_mask.astype(f32)[..., None])
    cm = (c_hidden.astype(f32) * c_mask.astype(f32)[..., None])

    # ---- c-side packing: global size-sorted, round-robin core deal ----
    creal = c_mask[:, 1:].astype(bool)            # [C, SC-1]
    n_real = creal.sum(1)
    order = np.argsort(-n_real, kind="stable")    # desc by unmasked count

    # ---- q-side packing ----
    qreal = q_mask[:, 1:].astype(bool)            # [Q, SQ-1]
    kq = qreal.sum(1)
    rows_q, rows_tok = np.nonzero(qreal)          # packed order: by query
    npk = len(rows_tok) + 1                       # + bhat row
    nqtile = max(1, -(-npk // 128))
    p = _make_plan(n_real[order], nqtile)
    p["order"] = order
    qtp = p["qtp"]

    qtok = qm[:, 1:, :].reshape(QTOK, H)
    qpack = np.zeros((qtp, H), f32)
    qpack[:npk - 1] = qtok[rows_q * (SQ - 1) + rows_tok]
    # row npk-1 stays zero -> projects to bhat; pad rows stay zero too
    qT = np.ascontiguousarray(qpack.T).reshape(HC, 128, qtp)

    Ssel = np.zeros((p["nqt"], 128, Q), f32)
    rr = np.arange(npk - 1)
    Ssel[rr // 128, rr % 128, rows_q] = 1.0
    Ssel[(npk - 1) // 128, (npk - 1) % 128, :] = (SQ - 1) - kq

    if SIM_BF16:
        import ml_dtypes
        Ssel = Ssel.astype(ml_dtypes.bfloat16)
    if PROJ_BF16:
        import ml_dtypes
        qT = qT.astype(ml_dtypes.bfloat16)

    qcls = np.ascontiguousarray(qm[:, 0, :])
    Wblk = np.ascontiguousarray(W.astype(f32)).reshape(HC, 128, D)
    if PROJ_BF16:
        Wblk = Wblk.astype(ml_dtypes.bfloat16)
    bb = np.ascontiguousarray(b.astype(f32)).reshape(D, 1)
    with np.errstate(divide="ignore"):
        invd = (1.0 / kq.astype(f32)).reshape(Q, 1).astype(f32)

    # per-doc packed token index rows (into tokens 1..SC-1), width-agnostic
    reals = [np.nonzero(creal[d])[0] for d in range(C)]
    pads = [np.nonzero(~creal[d])[0] for d in range(C)]

    cmt = cm[:, 1:, :].reshape(C * (SC - 1), H)
    in_maps = []
    for s in range(NCORES):
        dl = order[s::NCORES]                     # this core's docs, desc
        rows = np.empty(p["ctp"], np.int64)
        for (i0, dpm, nmm, Wg, ccoff, sloff) in p["plan"]:
            nd = dpm * nmm
            for jj in range(nd):
                d = int(dl[i0 + jj])
                r = reals[d]
                pad = pads[d][0] if len(pads[d]) else r[0]
                row = np.full(Wg, pad, np.int64)
                row[:min(len(r), Wg)] = r[:Wg]
                rows[ccoff + jj * Wg: ccoff + (jj + 1) * Wg] = d * (SC - 1) + row
        cpack = np.zeros((p["ctp_pad"], H), f32)
        cpack[:p["ctp"]] = cmt[rows]
        cTs = np.ascontiguousarray(cpack.T).reshape(HC, 128, p["ctp_pad"])
        if PROJ_BF16:
            import ml_dtypes
            cTs = cTs.astype(ml_dtypes.bfloat16)
        ccls = np.ascontiguousarray(cm[dl, 0, :])
        in_maps.append(dict(qT=qT, cT=cTs, qcls=qcls, ccls=ccls, Wb=Wblk,
                            bb=bb, Ssel=Ssel, invd=invd,
                            onesc=(np.ones((128, 1), f32) if not SIM_BF16 else
                                   np.ones((128, 1), "bfloat16" and __import__("ml_dtypes").bfloat16)),
                            onesr=np.ones((1, 128), f32)))
    return p, in_maps


def _run(p, in_maps, trace=False, tmpdir=None):
    from concourse import bass_utils
    if trace:
        _install_ntff_hook()
        bass_utils.upload_artifacts = lambda d: d
    nc = _get_program(p)
    return bass_utils.run_bass_kernel_spmd(
        nc, in_maps, core_ids=list(range(NCORES)), trace=trace, tmpdir=tmpdir)


def _gather(res, p):
    order = p["order"]
    sim = np.empty((Q, C), np.float32)
    c_pooled = np.empty((C, H), np.float32)
    for s in range(NCORES):
        dl = order[s::NCORES]
        sim[:, dl] = res.results[s]["osim"]
        c_pooled[dl] = res.results[s]["ocp"]
    q_pooled = res.results[0]["oqp"]
    return sim, q_pooled, c_pooled


def kernel(q_hidden, c_hidden, W, b, q_mask, c_mask):
    p, in_maps = _prepare(q_hidden, c_hidden, W, b, q_mask, c_mask)
    res = _run(p, in_maps, trace=False)
    return _gather(res, p)


# revision 58
# speedup vs baseline: 1.0211x; 1.0211x over previous
"""Trainium2 Bass kernel for the ColBERT MaxSim retrieval problem.

Computes, for fixed shapes Q=64,Sq=32,C=256,Sc=256,H=768,D=128:
    q_pooled = l2norm(q_hidden[:,0] * q_mask[:,0])            [Q,H]
    c_pooled = l2norm(c_hidden[:,0] * c_mask[:,0])            [C,H]
    q_col    = l2norm((q_hidden*q_mask)[:,1:] @ W + b)        [Q,31,D]
    c_col    = l2norm((c_hidden*c_mask)[:,1:] @ W + b)        [C,255,D]
    sim[q,c] = sum_i max_j (q_col[q,i] . c_col[c,j]) / sum(q_mask[q,1:])

Sharding: candidates C are split across 8 NeuronCores (32 docs each);
q tensors / W / b are replicated.  Device-side layout is "colT":
projected token vectors live as [D=128 partitions, tokens] so the
similarity matmul contracts over D on the partition axis.

Exact mask-aware packing (host-side marshalling, device does all FLOPs):
a masked token's projected vector is always bhat = l2norm(b) (the
hidden state is zeroed before the linear layer, bias added after).  So
 - c side: per doc keep only unmasked token columns, pad each doc to a
   fixed width W with a masked-token column (which projects to bhat) —
   exactly the candidates the reference max_j sees.  Docs with no
   masked tokens are padded with a repeated real token (max-idempotent).
 - q side: drop masked query-token rows; add ONE shared all-zero row
   (projects to bhat) whose selector weight for query q is
   (31 - k_q), the number of masked tokens, since each of them would
   have contributed max_j(bhat . c_j).
The final segmented sum over query tokens is a small selector matmul.
"""

import os
import sys
import types

import numpy as np

for _p in ("/opt/trn_rl_repo", os.path.expanduser("~/.axon_site/_ro/trn_rl_repo")):
    if os.path.isdir(_p) and _p not in sys.path:
        sys.path.insert(0, _p)

# ---- problem constants (hardcoded per harness contract) ----
Q, SQ, C, SC, H, D = 64, 32, 256, 256, 768, 128
NCORES = 8
CPC = C // NCORES            # 32 docs per core
QTOK = Q * (SQ - 1)          # 1984 query tokens (unpacked)
HC = H // 128                # 6 contraction chunks
PBLK = 512                   # projection block (col count per psum tile)

USE_F32R = True              # fast fp32 matmul mode (1 cycle/row at N>=256)
SIM_BF16 = True              # bf16 operands for the similarity/final matmuls
PROJ_BF16 = True             # bf16 hidden-states/W for the projection matmuls

_PROGS = {}


def _install_ntff_hook():
    try:
        from antenv import axon_hooks  # noqa: F401
        return
    except ImportError:
        pass
    try:
        import antenv
        from trn_agent_boot.trn_boot import _ntff_profile_via_ctypes
    except ImportError:
        return
    mod = types.ModuleType("antenv.axon_hooks")
    holder = [None]
    mod.set_axon_ntff_profile_hook = lambda h: holder.__setitem__(0, h)
    mod.get_axon_ntff_profile_hook = lambda: holder[0]
    sys.modules["antenv.axon_hooks"] = mod
    antenv.axon_hooks = mod
    so = "/opt/axon/libaxon_pjrt.so"
    if os.path.exists(so):
        mod.set_axon_ntff_profile_hook(_ntff_profile_via_ctypes(so))


def _make_plan(ns_sorted, nqtile):
    """Build the shared per-core reduce-group plan from the globally
    DESC-sorted unmasked-token counts (len C).  Core s owns sorted docs
    [s::NCORES]; group widths are set by the group's first (largest)
    doc, which is the global max across cores for that local position.

    Returns params dict with plan = list of groups
    (local_doc0, dpm, nmm, W, ccol_off, slot_off)."""
    plan = []
    i = 0          # core-local doc index (sorted order)
    ccoff = 0
    while i < CPC:
        Wg = int(ns_sorted[i * NCORES]) + 1
        Wg += Wg % 2
        Wg = min(SC - 1, Wg)
        dpm = min(4, 512 // Wg)
        nd = min(2 * dpm, CPC - i)
        nmm = -(-nd // dpm)
        if nmm * dpm != nd:        # keep groups rectangular
            nd = (nd // dpm) * dpm
            nmm = nd // dpm
            if nd == 0:
                dpm = CPC - i
                nd = dpm
                nmm = 1
        plan.append((i, dpm, nmm, Wg, ccoff, i))
        ccoff += nd * Wg
        i += nd
    ctp = ccoff
    ctp_pad = -(-ctp // PBLK) * PBLK
    qtp = nqtile * 128
    key = (nqtile,) + tuple((dpm, nmm, W) for _, dpm, nmm, W, _, _ in plan)
    return dict(plan=plan, ctp=ctp, ctp_pad=ctp_pad, nqt=nqtile, qtp=qtp,
                key=key)


def _build_program(p):
    import concourse.tile as tile
    from concourse import bacc, mybir

    f32 = mybir.dt.float32
    f32m = mybir.dt.float32r if USE_F32R else f32
    simdt = mybir.dt.bfloat16 if SIM_BF16 else f32m
    hdt = mybir.dt.bfloat16 if PROJ_BF16 else f32m
    AF = mybir.ActivationFunctionType
    AX = mybir.AxisListType

    plan, ctp_pad, nqt, qtp = p["plan"], p["ctp_pad"], p["nqt"], p["qtp"]

    nc = bacc.Bacc(
        "TRN2",
        target_bir_lowering=False,
        debug=False,
        enable_asserts=False,
        num_devices=NCORES,
    )

    qT = nc.dram_tensor("qT", [HC, 128, qtp], hdt, kind="ExternalInput").ap()
    cT = nc.dram_tensor("cT", [HC, 128, ctp_pad], hdt, kind="ExternalInput").ap()
    qcls = nc.dram_tensor("qcls", [Q, H], f32, kind="ExternalInput").ap()
    ccls = nc.dram_tensor("ccls", [CPC, H], f32, kind="ExternalInput").ap()
    Wb = nc.dram_tensor("Wb", [HC, 128, D], hdt, kind="ExternalInput").ap()
    bb = nc.dram_tensor("bb", [D, 1], f32, kind="ExternalInput").ap()
    Ssel = nc.dram_tensor("Ssel", [nqt, 128, Q], simdt, kind="ExternalInput").ap()
    invd = nc.dram_tensor("invd", [Q, 1], f32, kind="ExternalInput").ap()
    onesc = nc.dram_tensor("onesc", [128, 1], simdt, kind="ExternalInput").ap()
    onesr = nc.dram_tensor("onesr", [1, 128], f32m, kind="ExternalInput").ap()

    osim = nc.dram_tensor("osim", [Q, CPC], f32, kind="ExternalOutput").ap()
    oqp = nc.dram_tensor("oqp", [Q, H], f32, kind="ExternalOutput").ap()
    ocp = nc.dram_tensor("ocp", [CPC, H], f32, kind="ExternalOutput").ap()

    with tile.TileContext(nc) as tc:
        with tc.tile_pool(name="const", bufs=1) as const:
            w_sb = const.tile([128, HC * D], hdt)
            nc.sync.dma_start(w_sb[:].rearrange("p (k d) -> p k d", k=HC),
                              Wb.rearrange("k p d -> p k d"))
            b_sb = const.tile([D, 1], f32)
            nc.scalar.dma_start(b_sb[:], bb)
            ones_col = const.tile([128, 1], simdt)
            nc.scalar.dma_start(ones_col[:], onesc)
            ones_row = const.tile([1, 128], f32m)
            nc.scalar.dma_start(ones_row[:], onesr)
            s_sb = const.tile([128, nqt * Q], simdt)
            nc.sync.dma_start(s_sb[:].rearrange("p (t q) -> p t q", t=nqt),
                              Ssel.rearrange("t p q -> p t q"))
            invd_sb = const.tile([Q, 1], f32)
            nc.scalar.dma_start(invd_sb[:], invd)

            qcol = const.tile([128, qtp], simdt)
            ccol = const.tile([128, ctp_pad], simdt)
            md = const.tile([128, nqt * CPC], simdt)

            # Flat scope; PSUM budget 8 banks: y 2 + ss 1 + r/fin 1 + sim 4.
            with tc.tile_pool(name="hT", bufs=6) as hpool, \
                 tc.tile_pool(name="zp", bufs=4) as zpool, \
                 tc.tile_pool(name="rows", bufs=4) as rowpool, \
                 tc.tile_pool(name="ypsum", bufs=2, space="PSUM") as ypsum, \
                 tc.tile_pool(name="spsum", bufs=1, space="PSUM") as spsum, \
                 tc.tile_pool(name="rpsum", bufs=1, space="PSUM") as rpsum, \
                 tc.tile_pool(name="simpsum", bufs=2, space="PSUM") as simpsum:

                # PE warm-up burst: dense matmuls during the DMA ramp so the
                # HAM clock-gate releases (K=8/8) before projection work.
                warm = spsum.tile([128, 512], f32, tag="ss")
                for _ in range(24):
                    nc.tensor.matmul(warm[:], w_sb[:, :128], w_sb[:, :512],
                                     start=True, stop=True)

                def project(srcT, ntok, dst):
                    nblk = -(-ntok // PBLK)
                    for i in range(nblk):
                        o = i * PBLK
                        blk = min(PBLK, ntok - o)
                        h_sb = hpool.tile([128, HC, PBLK], hdt, tag="hT")
                        dma_eng = nc.sync if i % 2 == 0 else nc.gpsimd
                        dma_eng.dma_start(
                            h_sb[:, :, :blk],
                            srcT.rearrange("k p t -> p k t")[:, :, o:o + blk])
                        y_ps = ypsum.tile([128, 512], f32, tag="y")
                        for k in range(HC):
                            nc.tensor.matmul(
                                y_ps[:, :blk],
                                w_sb[:, k * D:(k + 1) * D],
                                h_sb[:, k, :blk],
                                start=(k == 0),
                                stop=(k == HC - 1),
                            )
                        # HAM keep-alive tied to this block's data
                        ka = ypsum.tile([128, 512], f32, tag="y")
                        nc.tensor.matmul(ka[:, :blk], w_sb[:, :128],
                                         h_sb[:, 0, :blk], start=True, stop=True)
                        # z = y + b  (bias per-partition), PSUM -> SBUF
                        z_sb = zpool.tile([128, PBLK], f32, tag="z")
                        nc.scalar.activation(z_sb[:, :blk], y_ps[:, :blk],
                                             AF.Identity, bias=b_sb[:], scale=1.0)
                        zsq = zpool.tile([128, PBLK], simdt, tag="zsq")
                        nc.vector.tensor_mul(zsq[:, :blk], z_sb[:, :blk],
                                             z_sb[:, :blk])
                        # sumsq over D via ones-matmul -> [1, blk]
                        ss_ps = spsum.tile([1, 512], f32, tag="ss")
                        nc.tensor.matmul(ss_ps[:, :blk], ones_col[:],
                                         zsq[:, :blk], start=True, stop=True)
                        # norm row [1,blk] (single ACT op: PSUM->SBUF + sqrt)
                        nrm = rowpool.tile([1, PBLK], f32m, tag="nrm")
                        nc.scalar.activation(nrm[:, :blk], ss_ps[:, :blk], AF.Sqrt)
                        # broadcast norm across partitions via K=1 matmul, then
                        # full-width reciprocal + multiply (128 lanes, not 1)
                        r_ps = rpsum.tile([128, 512], f32, tag="r")
                        nc.tensor.matmul(r_ps[:, :blk], ones_row[:],
                                         nrm[:, :blk], start=True, stop=True)
                        rcp = zpool.tile([128, PBLK], f32, tag="rcp")
                        nc.vector.reciprocal_approx_fast(rcp[:, :blk], r_ps[:, :blk])
                        nc.gpsimd.tensor_mul(dst[:, o:o + blk], z_sb[:, :blk],
                                              rcp[:, :blk])

                project(qT, qtp, qcol)

                # ---- pooled CLS outputs (natural layout, row-wise l2norm) ----
                def pooled(cls_ap, n, out_ap):
                    tcl = zpool.tile([n, H], f32, tag="cls%d" % n)
                    nc.scalar.dma_start(tcl[:], cls_ap)
                    sq = zpool.tile([n, H], f32, tag="clsq%d" % n)
                    nc.scalar.activation(sq[:], tcl[:], AF.Square)
                    s0 = rowpool.tile([n, 1], f32, tag="clss%d" % n)
                    nc.vector.reduce_sum(s0[:], sq[:], axis=AX.X)
                    s1 = rowpool.tile([n, 1], f32, tag="clss1%d" % n)
                    nc.scalar.activation(s1[:], s0[:], AF.Sqrt)
                    s2 = rowpool.tile([n, 1], f32, tag="clss2%d" % n)
                    nc.vector.tensor_scalar_max(s2[:], s1[:], 1e-12)
                    rr = rowpool.tile([n, 1], f32, tag="clsr%d" % n)
                    nc.vector.reciprocal(rr[:], s2[:])
                    oo = zpool.tile([n, H], f32, tag="clso%d" % n)
                    nc.vector.tensor_scalar_mul(oo[:], tcl[:], rr[:])
                    nc.sync.dma_start(out_ap, oo[:])

                pooled(qcls, Q, oqp)
                pooled(ccls, CPC, ocp)

                project(cT, ctp_pad, ccol)

                # ---- phase B: similarity matmuls + per-doc max ----
                for t in range(nqt):
                    qw = qcol[:, t * 128:(t + 1) * 128]
                    for gi, (i0, dpm, nmm, Wg, ccoff, sloff) in enumerate(plan):
                        if t >= 2 and gi % 2 == 0:
                            # HAM keep-alive: extra array activity so the
                            # clock-gate stays at K=8/8 through the
                            # reduce-bound tail (PE duty alone is too low).
                            ka = ypsum.tile([128, 512], f32, tag="y")
                            nc.tensor.matmul(ka[:, :512], qw,
                                             ccol[:, ccoff:ccoff + 512],
                                             start=True, stop=True)
                        ps = simpsum.tile([128, 1024], f32, tag="sim")
                        mw = dpm * Wg
                        for m in range(nmm):
                            nc.tensor.matmul(
                                ps[:, m * 512:m * 512 + mw], qw,
                                ccol[:, ccoff + m * mw: ccoff + (m + 1) * mw],
                                start=True, stop=True)
                        red_in = (
                            ps[:, :]
                            .rearrange("p (k x) -> p k x", k=2)[:, :nmm, :mw]
                            .rearrange("p k (d j) -> p k d j", j=Wg)
                        )
                        nd = dpm * nmm
                        nc.vector.reduce_max(
                            md[:, t * CPC + sloff: t * CPC + sloff + nd],
                            red_in, axis=AX.X,
                        )

                # ---- phase C: weighted sum over query tokens + scale ----
                fp = spsum.tile([Q, CPC], f32, tag="ss")
                for t in range(nqt):
                    nc.tensor.matmul(
                        fp[:],
                        s_sb[:, t * Q:(t + 1) * Q],
                        md[:, t * CPC: (t + 1) * CPC],
                        start=(t == 0), stop=(t == nqt - 1),
                    )
                so = zpool.tile([Q, CPC], f32, tag="so")
                nc.vector.tensor_scalar_mul(so[:], fp[:], invd_sb[:])
                nc.sync.dma_start(osim, so[:])

    nc.compile()
    return nc


def _get_program(p):
    key = p["key"]
    if key not in _PROGS:
        _PROGS[key] = _build_program(p)
    return _PROGS[key]


def _prepare(q_hidden, c_hidden, W, b, q_mask, c_mask):
    f32 = np.float32
    qm = (q_hidden.astype(f32) * q_mask.astype(f32)[..., None])
    cm = (c_hidden.astype(f32) * c_mask.astype(f32)[..., None])

    # ---- c-side packing: global size-sorted, round-robin core deal ----
    creal = c_mask[:, 1:].astype(bool)            # [C, SC-1]
    n_real = creal.sum(1)
    order = np.argsort(-n_real, kind="stable")    # desc by unmasked count

    # ---- q-side packing ----
    qreal = q_mask[:, 1:].astype(bool)            # [Q, SQ-1]
    kq = qreal.sum(1)
    rows_q, rows_tok = np.nonzero(qreal)          # packed order: by query
    npk = len(rows_tok) + 1                       # + bhat row
    nqtile = max(1, -(-npk // 128))
    p = _make_plan(n_real[order], nqtile)
    p["order"] = order
    qtp = p["qtp"]

    qtok = qm[:, 1:, :].reshape(QTOK, H)
    qpack = np.zeros((qtp, H), f32)
    qpack[:npk - 1] = qtok[rows_q * (SQ - 1) + rows_tok]
    # row npk-1 stays zero -> projects to bhat; pad rows stay zero too
    qT = np.ascontiguousarray(qpack.T).reshape(HC, 128, qtp)

    Ssel = np.zeros((p["nqt"], 128, Q), f32)
    rr = np.arange(npk - 1)
    Ssel[rr // 128, rr % 128, rows_q] = 1.0
    Ssel[(npk - 1) // 128, (npk - 1) % 128, :] = (SQ - 1) - kq

    if SIM_BF16:
        import ml_dtypes
        Ssel = Ssel.astype(ml_dtypes.bfloat16)
    if PROJ_BF16:
        import ml_dtypes
        qT = qT.astype(ml_dtypes.bfloat16)

    qcls = np.ascontiguousarray(qm[:, 0, :])
    Wblk = np.ascontiguousarray(W.astype(f32)).reshape(HC, 128, D)
    if PROJ_BF16:
        Wblk = Wblk.astype(ml_dtypes.bfloat16)
    bb = np.ascontiguousarray(b.astype(f32)).reshape(D, 1)
    with np.errstate(divide="ignore"):
        invd = (1.0 / kq.astype(f32)).reshape(Q, 1).astype(f32)

    # per-doc packed token index rows (into tokens 1..SC-1), width-agnostic
    reals = [np.nonzero(creal[d])[0] for d in range(C)]
    pads = [np.nonzero(~creal[d])[0] for d in range(C)]

    cmt = cm[:, 1:, :].reshape(C * (SC - 1), H)
    in_maps = []
    for s in range(NCORES):
        dl = order[s::NCORES]                     # this core's docs, desc
        rows = np.empty(p["ctp"], np.int64)
        for (i0, dpm, nmm, Wg, ccoff, sloff) in p["plan"]:
            nd = dpm * nmm
            for jj in range(nd):
                d = int(dl[i0 + jj])
                r = reals[d]
                pad = pads[d][0] if len(pads[d]) else r[0]
                row = np.full(Wg, pad, np.int64)
                row[:min(len(r), Wg)] = r[:Wg]
                rows[ccoff + jj * Wg: ccoff + (jj + 1) * Wg] = d * (SC - 1) + row
        cpack = np.zeros((p["ctp_pad"], H), f32)
        cpack[:p["ctp"]] = cmt[rows]
        cTs = np.ascontiguousarray(cpack.T).reshape(HC, 128, p["ctp_pad"])
        if PROJ_BF16:
            import ml_dtypes
            cTs = cTs.astype(ml_dtypes.bfloat16)
        ccls = np.ascontiguousarray(cm[dl, 0, :])
        in_maps.append(dict(qT=qT, cT=cTs, qcls=qcls, ccls=ccls, Wb=Wblk,
                            bb=bb, Ssel=Ssel, invd=invd,
                            onesc=(np.ones((128, 1), f32) if not SIM_BF16 else
                                   np.ones((128, 1), "bfloat16" and __import__("ml_dtypes").bfloat16)),
                            onesr=np.ones((1, 128), f32)))
    return p, in_maps


def _run(p, in_maps, trace=False, tmpdir=None):
    from concourse import bass_utils
    if trace:
        _install_ntff_hook()
        bass_utils.upload_artifacts = lambda d: d
    nc = _get_program(p)
    return bass_utils.run_bass_kernel_spmd(
        nc, in_maps, core_ids=list(range(NCORES)), trace=trace, tmpdir=tmpdir)


def _gather(res, p):
    order = p["order"]
    sim = np.empty((Q, C), np.float32)
    c_pooled = np.empty((C, H), np.float32)
    for s in range(NCORES):
        dl = order[s::NCORES]
        sim[:, dl] = res.results[s]["osim"]
        c_pooled[dl] = res.results[s]["ocp"]
    q_pooled = res.results[0]["oqp"]
    return sim, q_pooled, c_pooled


def kernel(q_hidden, c_hidden, W, b, q_mask, c_mask):
    p, in_maps = _prepare(q_hidden, c_hidden, W, b, q_mask, c_mask)
    res = _run(p, in_maps, trace=False)
    return _gather(res, p)
